# revision 4
# baseline (speedup 1.0000x reference)
"""HR2O_NL sparse-attention kernel for 8 Trainium2 NeuronCores.

Sharding: data-parallel over ROI groups (videos LPT-binpacked onto 8 cores,
whole groups stay local). Conv weights + GN params replicated. Each core runs
q/k/v 3x3 convs (bf16 matmuls, fp32 PSUM), per-position masked attention,
GroupNorm, relu, out-conv, residual — all on its ROI shard (padded to CAP=70).
"""
import sys, types
import numpy as np
import ml_dtypes

import concourse.bass as bass
import concourse.mybir as mybir
import concourse.tile as tile
from concourse.bass_utils import run_bass_kernel_spmd

BF = mybir.dt.bfloat16
F32 = mybir.dt.float32
CAP = 70          # padded ROIs per core
NB = 7            # roi blocks of 10
C = 512
P = 49            # 7x7 positions
NCORE = 8
NPOS = CAP * P    # 3430


def _install_profhook():
    if 'antenv.axon_hooks' in sys.modules:
        return
    try:
        from trn_agent_boot.trn_boot import _ntff_profile_via_ctypes
        hook = _ntff_profile_via_ctypes('/opt/axon/libaxon_pjrt.so')
    except Exception:
        hook = None
    m = types.ModuleType('antenv.axon_hooks')
    m.get_axon_ntff_profile_hook = lambda: hook
    sys.modules['antenv.axon_hooks'] = m


def _walk_blocks(bb):
    yield bb
    for inner in getattr(bb, 'blocks', []) or []:
        yield from _walk_blocks(inner)


def _split_multiwait(nc):
    # this walrus build accepts one sync wait per instruction
    fn = nc.m.functions[0]
    for bb in list(_walk_blocks(fn)):
        insts = getattr(bb, 'instructions', None)
        if not insts:
            continue
        new_list, changed = [], False
        for inst in insts:
            si = inst.sync_info
            if si is not None and si.on_wait is not None and len(si.on_wait) > 1:
                waits = list(si.on_wait)
                for j, w in enumerate(waits[:-1]):
                    d = mybir.InstDrain(name=f"{inst.name}_ws{j}", ins=[], outs=[])
                    d.engine = inst.engine
                    d.sync_info = mybir.SyncInfo(on_wait=[w], on_update=[])
                    new_list.append(d)
                si.on_wait = [waits[-1]]
                changed = True
            new_list.append(inst)
        if changed:
            insts[:] = new_list


_NC_CACHE = {}


def _build():
    if 'nc' in _NC_CACHE:
        return _NC_CACHE['nc']
    nc = bass.Bass("TRN2", target_bir_lowering=False, debug=False, num_devices=NCORE)
    xp_d = nc.dram_tensor("xp", [4, 128, CAP * 81], BF, kind="ExternalInput")
    xint_d = nc.dram_tensor("xint", [4, 128, NPOS], F32, kind="ExternalInput")
    wq_d = nc.dram_tensor("wq", [4, 128, 9, 4, 128], BF, kind="ExternalInput")
    wk_d = nc.dram_tensor("wk", [4, 128, 9, 4, 128], BF, kind="ExternalInput")
    wv_d = nc.dram_tensor("wv", [4, 128, 9, 4, 128], BF, kind="ExternalInput")
    wo_d = nc.dram_tensor("wo", [4, 128, 9, 4, 128], BF, kind="ExternalInput")
    mask_d = nc.dram_tensor("mask", [CAP, CAP], F32, kind="ExternalInput")
    y_d = nc.dram_tensor("y", [4, 128, NPOS], F32, kind="ExternalOutput")
    v_dram = nc.dram_tensor("v_sc", [CAP, 4, 128, P], BF)
    sc1 = nc.dram_tensor("sc1", [NPOS], F32)   # recip bounce
    sc2 = nc.dram_tensor("sc2", [NPOS], F32)   # rstd bounce
    sc3 = nc.dram_tensor("sc3", [NPOS], F32)   # negB bounce

    def conv_rhs(xt, blk, tap):
        dy, dx = tap // 3, tap % 3
        return bass.AP(tensor=xt.tensor, offset=xt.offset + blk * 810 + dy * 9 + dx,
                       ap=[xt.ap[0], [81, 10], [9, 7], [1, 7]])

    def bcast_read(handle, n):
        base = handle[:]
        return bass.AP(tensor=base.tensor, offset=0, ap=[[0, 128], [1, n]])

    with tile.TileContext(nc) as tc:
        with tc.tile_pool(name="persist", bufs=1) as pp:
            attw = pp.tile([70, P, 70], BF, name="attw")
            virt = [pp.tile([128, P, 70], F32, name=f"virt{t}") for t in range(4)]
            qkp_cm = tc.tile_pool(name="qk", bufs=1)
            qkp = qkp_cm.__enter__()
            q_s = [qkp.tile([128, NPOS], BF, name=f"q{t}") for t in range(4)]
            k_s = [qkp.tile([128, NPOS], BF, name=f"k{t}") for t in range(4)]

            # ---------------- phase 1: q,k,v convs ----------------
            with (
                tc.tile_pool(name="p1", bufs=1) as p1,
                tc.tile_pool(name="wts", bufs=2) as wts,
                tc.tile_pool(name="vst", bufs=3) as vst,
                tc.tile_pool(name="ps1", bufs=6, space="PSUM") as ps1,
            ):
                xt = [p1.tile([128, CAP * 81], BF, name=f"xp{c}") for c in range(4)]
                for c in range(4):
                    nc.sync.dma_start(out=xt[c][:], in_=xp_d[c])
                for wd, dst in ((wq_d, q_s), (wk_d, k_s), (wv_d, None)):
                    for cto in range(4):
                        wt = wts.tile([128, 4, 9, 128], BF, name="wt", tag="wt")
                        for ci in range(4):
                            srcap = bass.AP(
                                tensor=wd[:].tensor, offset=ci * 589824 + cto * 128,
                                ap=[[4608, 128], [512, 9], [1, 128]])
                            nc.sync.dma_start(out=wt[:, ci, :, :], in_=srcap)
                        for blk in range(NB):
                            acc = ps1.tile([128, 490], F32, name="acc", tag="acc")
                            fi = True
                            for ci in range(4):
                                for tap in range(9):
                                    nc.tensor.matmul(acc[:], wt[:, ci, tap, :],
                                                     conv_rhs(xt[ci], blk, tap),
                                                     start=fi, stop=(ci == 3 and tap == 8))
                                    fi = False
                            if dst is not None:
                                nc.vector.tensor_copy(
                                    dst[cto][:, blk * 490:(blk + 1) * 490], acc[:])
                            else:
                                vs = vst.tile([128, 490], BF, name="vs", tag="vs")
                                nc.vector.tensor_copy(vs[:], acc[:])
                                dstap = bass.AP(
                                    tensor=v_dram[:].tensor,
                                    offset=(blk * 10 * 4 + cto) * 128 * P,
                                    ap=[[P, 128], [4 * 128 * P, 10], [1, P]])
                                nc.sync.dma_start(out=dstap, in_=vs[:])

            # ---------------- phase 2a: QK^T + mask + exp ----------------
            with (
                tc.tile_pool(name="p2a", bufs=1) as p2a,
                tc.tile_pool(name="ps2", bufs=4, space="PSUM") as ps2,
            ):
                mask_t = p2a.tile([CAP, CAP], F32, name="mask")
                nc.sync.dma_start(out=mask_t[:], in_=mask_d[:])
                attf = p2a.tile([70, P, 70], F32, name="attf")
                for p in range(P):
                    aps = ps2.tile([70, 70], F32, name="aps", tag="aps")
                    for ct in range(4):
                        lhsT = bass.AP(tensor=k_s[ct].tensor, offset=k_s[ct].offset + p,
                                       ap=[k_s[ct].ap[0], [P, 70]])
                        rhs = bass.AP(tensor=q_s[ct].tensor, offset=q_s[ct].offset + p,
                                      ap=[q_s[ct].ap[0], [P, 70]])
                        nc.tensor.matmul(aps[:], lhsT, rhs, start=(ct == 0), stop=(ct == 3))
                    nc.vector.tensor_add(attf[:, p, :], aps[:], mask_t[:70, :70])
                nc.scalar.activation(
                    attw.rearrange("a b c -> a (b c)"),
                    attf.rearrange("a b c -> a (b c)"),
                    func=mybir.ActivationFunctionType.Exp)
            qkp_cm.__exit__(None, None, None)

            # ---------------- phase 2b: rowsum, AV, divide ----------------
            with (
                tc.tile_pool(name="p2b", bufs=1) as p2b,
                tc.tile_pool(name="stats", bufs=1) as stp,
                tc.tile_pool(name="ps3", bufs=4, space="PSUM") as ps3,
                tc.tile_pool(name="ps4", bufs=2, space="PSUM") as ps4,
            ):
                ones_t = p2b.tile([70, 1], BF, name="ones")
                nc.vector.memset(ones_t[:], 1.0)
                rsum = stp.tile([1, NPOS], F32, name="rsum", tag="st")
                for pc in range(7):
                    op = ps4.tile([1, 490], F32, name="op", tag="op")
                    nc.tensor.matmul(
                        op[:], ones_t[:],
                        attw[:, pc * 7:(pc + 1) * 7, :].rearrange("a b c -> a (b c)"),
                        start=True, stop=True)
                    nc.vector.tensor_copy(rsum[:, pc * 490:(pc + 1) * 490], op[:])
                nc.vector.reciprocal(rsum[:], rsum[:])
                nc.sync.dma_start(out=sc1[:], in_=rsum[0:1, :])
                recip_b = p2b.tile([128, NPOS], F32, name="recipb")
                nc.sync.dma_start(out=recip_b[:], in_=bcast_read(sc1, NPOS))
                vth = p2b.tile([70, 2, 128, P], BF, name="vth", tag="vth")
                for h in range(2):
                    if h == 1:
                        vth = p2b.tile([70, 2, 128, P], BF, name="vth2", tag="vth")
                    nc.sync.dma_start(out=vth[:], in_=v_dram[:, h * 2:h * 2 + 2])
                    for ctp in range(2):
                        ct = h * 2 + ctp
                        for p in range(P):
                            av = ps3.tile([128, 70], F32, name="av", tag="av")
                            nc.tensor.matmul(av[:], vth[:, ctp, :, p], attw[:, p, :],
                                             start=True, stop=True)
                            nc.vector.tensor_copy(virt[ct][:, p, :], av[:])
                for ct in range(4):
                    vf = virt[ct].rearrange("a b c -> a (b c)")
                    nc.vector.tensor_mul(vf, vf, recip_b[:])

                # ---- GroupNorm stats (per-i over c,p) ----
                vbfp = p2b.tile([128, NPOS], BF, name="vbf", tag="vbf")
                s1 = stp.tile([1, NPOS], F32, name="s1", tag="st")
                s2 = stp.tile([1, NPOS], F32, name="s2", tag="st2")
                onesf = p2b.tile([128, 1], BF, name="onesf")
                nc.vector.memset(onesf[:], 1.0)
                for which, sdst in ((0, s1), (1, s2)):
                    for chunk in range(7):
                        op = ps4.tile([1, 490], F32, name="op2", tag="op")
                        for ct in range(4):
                            vf = virt[ct].rearrange("a b c -> a (b c)")
                            seg = vf[:, chunk * 490:(chunk + 1) * 490]
                            if which == 0:
                                nc.vector.tensor_copy(vbfp[:, chunk * 490:(chunk + 1) * 490], seg)
                            else:
                                nc.vector.tensor_mul(vbfp[:, chunk * 490:(chunk + 1) * 490], seg, seg)
                            nc.tensor.matmul(op[:], onesf[:],
                                             vbfp[:, chunk * 490:(chunk + 1) * 490],
                                             start=(ct == 0), stop=(ct == 3))
                        nc.vector.tensor_copy(sdst[:, chunk * 490:(chunk + 1) * 490], op[:])

                s1i = p2b.tile([1, 70], F32, name="s1i")
                s2i = p2b.tile([1, 70], F32, name="s2i")
                for src, dsti in ((s1, s1i), (s2, s2i)):
                    v3 = bass.AP(tensor=src.tensor, offset=src.offset,
                                 ap=[src.ap[0], [1, 70], [70, P]])
                    nc.vector.reduce_sum(dsti[:], v3, axis=mybir.AxisListType.X)
                inv_n = 1.0 / (C * P)
                mean_r = p2b.tile([1, 70], F32, name="meanr")
                var_r = p2b.tile([1, 70], F32, name="varr")
                nc.vector.tensor_scalar_mul(mean_r[:], s1i[:], inv_n)
                nc.vector.tensor_scalar_mul(var_r[:], s2i[:], inv_n)
                msq = p2b.tile([1, 70], F32, name="msq")
                nc.vector.tensor_mul(msq[:], mean_r[:], mean_r[:])
                nc.vector.tensor_sub(var_r[:], var_r[:], msq[:])
                eps_t = p2b.tile([1, 1], F32, name="eps")
                nc.vector.memset(eps_t[:], 1e-5)
                nc.scalar.activation(var_r[:], var_r[:],
                                     func=mybir.ActivationFunctionType.Sqrt,
                                     bias=eps_t[:], scale=1.0)
                nc.vector.reciprocal(var_r[:], var_r[:])   # rstd
                negb_r = p2b.tile([1, 70], F32, name="negbr")
                nc.vector.tensor_mul(negb_r[:], mean_r[:], var_r[:])
                nc.vector.tensor_scalar_mul(negb_r[:], negb_r[:], -1.0)
                rstd_f = stp.tile([1, NPOS], F32, name="rstdf", tag="st")
                negb_f = stp.tile([1, NPOS], F32, name="negbf", tag="st2")
                for p in range(P):
                    nc.vector.tensor_copy(rstd_f[:, p * 70:(p + 1) * 70], var_r[:])
                    nc.vector.tensor_copy(negb_f[:, p * 70:(p + 1) * 70], negb_r[:])
                nc.sync.dma_start(out=sc2[:], in_=rstd_f[0:1, :])
                nc.sync.dma_start(out=sc3[:], in_=negb_f[0:1, :])

            # ---------------- phase 3: normalize, relu, out conv, residual --------
            with (
                tc.tile_pool(name="p3", bufs=1) as p3,
                tc.tile_pool(name="wts3", bufs=2) as wts3,
                tc.tile_pool(name="xin3", bufs=3) as xin3,
                tc.tile_pool(name="ost", bufs=3) as ost,
                tc.tile_pool(name="ps5", bufs=6, space="PSUM") as ps5,
            ):
                rstd_b = p3.tile([128, NPOS], F32, name="rstdb")
                negb_b = p3.tile([128, NPOS], F32, name="negbb")
                nc.sync.dma_start(out=rstd_b[:], in_=bcast_read(sc2, NPOS))
                nc.sync.dma_start(out=negb_b[:], in_=bcast_read(sc3, NPOS))
                rp = [p3.tile([128, CAP * 81], BF, name=f"rp{c}") for c in range(4)]
                for ct in range(4):
                    nc.vector.memset(rp[ct][:], 0.0)
                    vf = virt[ct].rearrange("a b c -> a (b c)")
                    nc.vector.tensor_mul(vf, vf, rstd_b[:])
                    nc.vector.tensor_add(vf, vf, negb_b[:])
                    dst = bass.AP(tensor=rp[ct].tensor, offset=rp[ct].offset + 10,
                                  ap=[rp[ct].ap[0], [9, 7], [1, 7], [81, 70]])
                    src = virt[ct].rearrange("a (y x) i -> a y x i", y=7)
                    nc.scalar.activation(dst, src,
                                         func=mybir.ActivationFunctionType.Relu)
                for cto in range(4):
                    wt = wts3.tile([128, 4, 9, 128], BF, name="wt3", tag="wt3")
                    for ci in range(4):
                        srcap = bass.AP(
                            tensor=wo_d[:].tensor, offset=ci * 589824 + cto * 128,
                            ap=[[4608, 128], [512, 9], [1, 128]])
                        nc.sync.dma_start(out=wt[:, ci, :, :], in_=srcap)
                    for blk in range(NB):
                        xit = xin3.tile([128, 490], F32, name="xi", tag="xi")
                        nc.sync.dma_start(
                            out=xit[:], in_=xint_d[cto][:, blk * 490:(blk + 1) * 490])
                        acc = ps5.tile([128, 490], F32, name="acc3", tag="acc3")
                        fi = True
                        for ci in range(4):
                            for tap in range(9):
                                nc.tensor.matmul(acc[:], wt[:, ci, tap, :],
                                                 conv_rhs(rp[ci], blk, tap),
                                                 start=fi, stop=(ci == 3 and tap == 8))
                                fi = False
                        o = ost.tile([128, 490], F32, name="o", tag="o")
                        nc.vector.tensor_add(o[:], acc[:], xit[:])
                        nc.sync.dma_start(
                            out=y_d[cto][:, blk * 490:(blk + 1) * 490], in_=o[:])

    _split_multiwait(nc)
    _NC_CACHE['nc'] = nc
    return nc


def _shard(rois):
    vid = rois[:, 0].astype(np.int64)
    sizes = np.bincount(vid, minlength=32)
    order = np.argsort(-sizes, kind='stable')
    loads = np.zeros(NCORE, np.int64)
    v2c = np.zeros(32, np.int64)
    for v in order:
        c = int(np.argmin(loads))
        loads[c] += sizes[v]
        v2c[v] = c
    core_of_roi = v2c[vid]
    idxs = [np.nonzero(core_of_roi == c)[0] for c in range(NCORE)]
    for ix in idxs:
        assert len(ix) <= CAP, f"core load {len(ix)} exceeds CAP={CAP}"
    return idxs, vid


def kernel(x, rois, w_q, w_k, w_v, w_out, gamma, beta):
    _install_profhook()
    nc = _build()
    x = np.asarray(x, np.float32)
    rois = np.asarray(rois)
    assert np.allclose(np.asarray(gamma), 1.0) and np.allclose(np.asarray(beta), 0.0), \
        "kernel folds GN affine assuming gamma=1, beta=0"
    idxs, vid = _shard(rois)

    def wprep(w, scale=1.0):
        # [co, ci, 1, 3, 3] -> [ci(4,128), tap, co(4,128)] bf16
        a = (np.asarray(w, np.float32)[:, :, 0] * scale).transpose(1, 2, 3, 0)
        return np.ascontiguousarray(
            a.reshape(4, 128, 9, 4, 128)).astype(ml_dtypes.bfloat16)

    wq = wprep(w_q, 1.0 / np.sqrt(np.float32(C)))
    wk, wv, wo = wprep(w_k), wprep(w_v), wprep(w_out)

    in_maps = []
    for c in range(NCORE):
        ix = idxs[c]
        n = len(ix)
        xpad = np.zeros((CAP, C, 9, 9), np.float32)
        xpad[:n, :, 1:8, 1:8] = x[ix, :, 0]
        xp = np.ascontiguousarray(
            xpad.transpose(1, 0, 2, 3).reshape(4, 128, CAP * 81)
        ).astype(ml_dtypes.bfloat16)
        xi = np.zeros((CAP, C, P), np.float32)
        xi[:n] = x[ix, :, 0].reshape(n, C, P)
        xint = np.ascontiguousarray(xi.transpose(1, 0, 2).reshape(4, 128, NPOS))
        ids = np.full(CAP, -1, np.int64)
        ids[:n] = vid[ix]
        ids[n:] = 1000 + np.arange(CAP - n)
        mask = np.where(ids[:, None] == ids[None, :], 0.0, -1e30).astype(np.float32)
        in_maps.append(dict(xp=xp, xint=xint, wq=wq, wk=wk, wv=wv, wo=wo, mask=mask))

    res = run_bass_kernel_spmd(nc, in_maps, list(range(NCORE)))
    kernel.last_exec_ns = res.exec_time_ns

    out = np.empty((512, C, 1, 7, 7), np.float32)
    for c in range(NCORE):
        ix = idxs[c]
        n = len(ix)
        yc = res.results[c]["y"].reshape(C, CAP, P).transpose(1, 0, 2)
        out[ix] = yc[:n].reshape(n, C, 1, 7, 7)
    return out


# revision 5
# speedup vs baseline: 1.0058x; 1.0058x over previous
"""HR2O_NL sparse-attention kernel for 8 Trainium2 NeuronCores.

Sharding: data-parallel over ROI groups (videos LPT-binpacked onto 8 cores,
whole groups stay local). Conv weights + GN params replicated. Each core runs
q/k/v 3x3 convs (bf16 matmuls, fp32 PSUM), per-position masked attention,
GroupNorm, relu, out-conv, residual — all on its ROI shard (padded to CAP=70).
"""
import sys, types
import numpy as np
import ml_dtypes

import concourse.bass as bass
import concourse.mybir as mybir
import concourse.tile as tile
from concourse.bass_utils import run_bass_kernel_spmd

BF = mybir.dt.bfloat16
F32 = mybir.dt.float32
CAP = 70          # padded ROIs per core
NB = 7            # roi blocks of 10
C = 512
P = 49            # 7x7 positions
NCORE = 8
NPOS = CAP * P    # 3430


def _install_profhook():
    if 'antenv.axon_hooks' in sys.modules:
        return
    try:
        from trn_agent_boot.trn_boot import _ntff_profile_via_ctypes
        hook = _ntff_profile_via_ctypes('/opt/axon/libaxon_pjrt.so')
    except Exception:
        hook = None
    m = types.ModuleType('antenv.axon_hooks')
    m.get_axon_ntff_profile_hook = lambda: hook
    sys.modules['antenv.axon_hooks'] = m


def _walk_blocks(bb):
    yield bb
    for inner in getattr(bb, 'blocks', []) or []:
        yield from _walk_blocks(inner)


def _split_multiwait(nc):
    # this walrus build accepts one sync wait per instruction
    fn = nc.m.functions[0]
    for bb in list(_walk_blocks(fn)):
        insts = getattr(bb, 'instructions', None)
        if not insts:
            continue
        new_list, changed = [], False
        for inst in insts:
            si = inst.sync_info
            if si is not None and si.on_wait is not None and len(si.on_wait) > 1:
                waits = list(si.on_wait)
                for j, w in enumerate(waits[:-1]):
                    d = mybir.InstDrain(name=f"{inst.name}_ws{j}", ins=[], outs=[])
                    d.engine = inst.engine
                    d.sync_info = mybir.SyncInfo(on_wait=[w], on_update=[])
                    new_list.append(d)
                si.on_wait = [waits[-1]]
                changed = True
            new_list.append(inst)
        if changed:
            insts[:] = new_list


_NC_CACHE = {}


def _build():
    if 'nc' in _NC_CACHE:
        return _NC_CACHE['nc']
    nc = bass.Bass("TRN2", target_bir_lowering=False, debug=False, num_devices=NCORE)
    xp_d = nc.dram_tensor("xp", [4, 128, CAP * 81], BF, kind="ExternalInput")
    xint_d = nc.dram_tensor("xint", [4, 128, NPOS], F32, kind="ExternalInput")
    wq_d = nc.dram_tensor("wq", [4, 128, 9, 4, 128], BF, kind="ExternalInput")
    wk_d = nc.dram_tensor("wk", [4, 128, 9, 4, 128], BF, kind="ExternalInput")
    wv_d = nc.dram_tensor("wv", [4, 128, 9, 4, 128], BF, kind="ExternalInput")
    wo_d = nc.dram_tensor("wo", [4, 128, 9, 4, 128], BF, kind="ExternalInput")
    mask_d = nc.dram_tensor("mask", [CAP, CAP], F32, kind="ExternalInput")
    y_d = nc.dram_tensor("y", [4, 128, NPOS], F32, kind="ExternalOutput")
    v_dram = nc.dram_tensor("v_sc", [CAP, 4, 128, P], BF)
    sc1 = nc.dram_tensor("sc1", [NPOS], F32)   # recip bounce
    sc2 = nc.dram_tensor("sc2", [NPOS], F32)   # rstd bounce
    sc3 = nc.dram_tensor("sc3", [NPOS], F32)   # negB bounce

    def conv_rhs(xt, blk, tap):
        dy, dx = tap // 3, tap % 3
        return bass.AP(tensor=xt.tensor, offset=xt.offset + blk * 810 + dy * 9 + dx,
                       ap=[xt.ap[0], [81, 10], [9, 7], [1, 7]])

    def bcast_read(handle, n):
        base = handle[:]
        return bass.AP(tensor=base.tensor, offset=0, ap=[[0, 128], [1, n]])

    with tile.TileContext(nc) as tc:
        with tc.tile_pool(name="persist", bufs=1) as pp:
            attw = pp.tile([70, P, 70], BF, name="attw")
            virt = [pp.tile([128, P, 70], F32, name=f"virt{t}") for t in range(4)]
            qkp_cm = tc.tile_pool(name="qk", bufs=1)
            qkp = qkp_cm.__enter__()
            q_s = [qkp.tile([128, NPOS], BF, name=f"q{t}") for t in range(4)]
            k_s = [qkp.tile([128, NPOS], BF, name=f"k{t}") for t in range(4)]

            # ---------------- phase 1: q,k,v convs ----------------
            with (
                tc.tile_pool(name="p1", bufs=1) as p1,
                tc.tile_pool(name="wts", bufs=2) as wts,
                tc.tile_pool(name="vst", bufs=3) as vst,
                tc.tile_pool(name="ps1", bufs=8, space="PSUM") as ps1,
            ):
                xt = [p1.tile([128, CAP * 81], BF, name=f"xp{c}") for c in range(4)]
                for c in range(4):
                    nc.sync.dma_start(out=xt[c][:], in_=xp_d[c])
                for wd, dst in ((wq_d, q_s), (wk_d, k_s), (wv_d, None)):
                    for cto in range(4):
                        wt = wts.tile([128, 4, 9, 128], BF, name="wt", tag="wt")
                        for ci in range(4):
                            srcap = bass.AP(
                                tensor=wd[:].tensor, offset=ci * 589824 + cto * 128,
                                ap=[[4608, 128], [512, 9], [1, 128]])
                            nc.sync.dma_start(out=wt[:, ci, :, :], in_=srcap)
                        for blk in range(NB):
                            acc = ps1.tile([128, 490], F32, name="acc", tag="acc")
                            fi = True
                            for ci in range(4):
                                for tap in range(9):
                                    nc.tensor.matmul(acc[:], wt[:, ci, tap, :],
                                                     conv_rhs(xt[ci], blk, tap),
                                                     start=fi, stop=(ci == 3 and tap == 8))
                                    fi = False
                            if dst is not None:
                                nc.vector.tensor_copy(
                                    dst[cto][:, blk * 490:(blk + 1) * 490], acc[:])
                            else:
                                vs = vst.tile([128, 490], BF, name="vs", tag="vs")
                                nc.vector.tensor_copy(vs[:], acc[:])
                                dstap = bass.AP(
                                    tensor=v_dram[:].tensor,
                                    offset=(blk * 10 * 4 + cto) * 128 * P,
                                    ap=[[P, 128], [4 * 128 * P, 10], [1, P]])
                                nc.sync.dma_start(out=dstap, in_=vs[:])

            # ---------------- phase 2a: QK^T + mask + exp ----------------
            with (
                tc.tile_pool(name="p2a", bufs=1) as p2a,
                tc.tile_pool(name="ps2", bufs=4, space="PSUM") as ps2,
            ):
                mask_t = p2a.tile([CAP, CAP], F32, name="mask")
                nc.sync.dma_start(out=mask_t[:], in_=mask_d[:])
                mask7 = p2a.tile([70, 7, 70], F32, name="mask7")
                for r in range(7):
                    nc.vector.tensor_copy(mask7[:, r, :], mask_t[:70, :70])
                attf = p2a.tile([70, P, 70], F32, name="attf")
                for pg in range(7):
                    aps = ps2.tile([70, 490], F32, name="aps", tag="aps")
                    for pp in range(7):
                        p = pg * 7 + pp
                        for ct in range(4):
                            lhsT = bass.AP(tensor=k_s[ct].tensor, offset=k_s[ct].offset + p,
                                           ap=[k_s[ct].ap[0], [P, 70]])
                            rhs = bass.AP(tensor=q_s[ct].tensor, offset=q_s[ct].offset + p,
                                          ap=[q_s[ct].ap[0], [P, 70]])
                            nc.tensor.matmul(aps[:, pp * 70:(pp + 1) * 70], lhsT, rhs,
                                             start=(ct == 0), stop=(ct == 3))
                    nc.vector.tensor_add(
                        attf[:, pg * 7:(pg + 1) * 7, :].rearrange("a b c -> a (b c)"),
                        aps[:], mask7.rearrange("a b c -> a (b c)"))
                nc.scalar.activation(
                    attw.rearrange("a b c -> a (b c)"),
                    attf.rearrange("a b c -> a (b c)"),
                    func=mybir.ActivationFunctionType.Exp)
            qkp_cm.__exit__(None, None, None)

            # ---------------- phase 2b: rowsum, AV, divide ----------------
            with (
                tc.tile_pool(name="p2b", bufs=1) as p2b,
                tc.tile_pool(name="stats", bufs=1) as stp,
                tc.tile_pool(name="ps3", bufs=4, space="PSUM") as ps3,
                tc.tile_pool(name="ps4", bufs=2, space="PSUM") as ps4,
            ):
                ones_t = p2b.tile([70, 1], BF, name="ones")
                nc.vector.memset(ones_t[:], 1.0)
                rsum = stp.tile([1, NPOS], F32, name="rsum", tag="st")
                for pc in range(7):
                    op = ps4.tile([1, 490], F32, name="op", tag="op")
                    nc.tensor.matmul(
                        op[:], ones_t[:],
                        attw[:, pc * 7:(pc + 1) * 7, :].rearrange("a b c -> a (b c)"),
                        start=True, stop=True)
                    nc.vector.tensor_copy(rsum[:, pc * 490:(pc + 1) * 490], op[:])
                nc.vector.reciprocal(rsum[:], rsum[:])
                nc.sync.dma_start(out=sc1[:], in_=rsum[0:1, :])
                recip_b = p2b.tile([128, NPOS], F32, name="recipb")
                nc.sync.dma_start(out=recip_b[:], in_=bcast_read(sc1, NPOS))
                vth = p2b.tile([70, 2, 128, P], BF, name="vth", tag="vth")
                for h in range(2):
                    if h == 1:
                        vth = p2b.tile([70, 2, 128, P], BF, name="vth2", tag="vth")
                    nc.sync.dma_start(out=vth[:], in_=v_dram[:, h * 2:h * 2 + 2])
                    for ctp in range(2):
                        ct = h * 2 + ctp
                        for pg in range(7):
                            av = ps3.tile([128, 490], F32, name="av", tag="av")
                            for pp in range(7):
                                p = pg * 7 + pp
                                nc.tensor.matmul(av[:, pp * 70:(pp + 1) * 70],
                                                 vth[:, ctp, :, p], attw[:, p, :],
                                                 start=True, stop=True)
                            nc.vector.tensor_copy(
                                virt[ct][:, pg * 7:(pg + 1) * 7, :].rearrange("a b c -> a (b c)"),
                                av[:])
                for ct in range(4):
                    vf = virt[ct].rearrange("a b c -> a (b c)")
                    nc.vector.tensor_mul(vf, vf, recip_b[:])

                # ---- GroupNorm stats (per-i over c,p) ----
                vbfp = p2b.tile([128, NPOS], BF, name="vbf", tag="vbf")
                s1 = stp.tile([1, NPOS], F32, name="s1", tag="st")
                s2 = stp.tile([1, NPOS], F32, name="s2", tag="st2")
                onesf = p2b.tile([128, 1], BF, name="onesf")
                nc.vector.memset(onesf[:], 1.0)
                for which, sdst in ((0, s1), (1, s2)):
                    for chunk in range(7):
                        op = ps4.tile([1, 490], F32, name="op2", tag="op")
                        for ct in range(4):
                            vf = virt[ct].rearrange("a b c -> a (b c)")
                            seg = vf[:, chunk * 490:(chunk + 1) * 490]
                            if which == 0:
                                nc.vector.tensor_copy(vbfp[:, chunk * 490:(chunk + 1) * 490], seg)
                            else:
                                nc.vector.tensor_mul(vbfp[:, chunk * 490:(chunk + 1) * 490], seg, seg)
                            nc.tensor.matmul(op[:], onesf[:],
                                             vbfp[:, chunk * 490:(chunk + 1) * 490],
                                             start=(ct == 0), stop=(ct == 3))
                        nc.vector.tensor_copy(sdst[:, chunk * 490:(chunk + 1) * 490], op[:])

                s1i = p2b.tile([1, 70], F32, name="s1i")
                s2i = p2b.tile([1, 70], F32, name="s2i")
                for src, dsti in ((s1, s1i), (s2, s2i)):
                    v3 = bass.AP(tensor=src.tensor, offset=src.offset,
                                 ap=[src.ap[0], [1, 70], [70, P]])
                    nc.vector.reduce_sum(dsti[:], v3, axis=mybir.AxisListType.X)
                inv_n = 1.0 / (C * P)
                mean_r = p2b.tile([1, 70], F32, name="meanr")
                var_r = p2b.tile([1, 70], F32, name="varr")
                nc.vector.tensor_scalar_mul(mean_r[:], s1i[:], inv_n)
                nc.vector.tensor_scalar_mul(var_r[:], s2i[:], inv_n)
                msq = p2b.tile([1, 70], F32, name="msq")
                nc.vector.tensor_mul(msq[:], mean_r[:], mean_r[:])
                nc.vector.tensor_sub(var_r[:], var_r[:], msq[:])
                eps_t = p2b.tile([1, 1], F32, name="eps")
                nc.vector.memset(eps_t[:], 1e-5)
                nc.scalar.activation(var_r[:], var_r[:],
                                     func=mybir.ActivationFunctionType.Sqrt,
                                     bias=eps_t[:], scale=1.0)
                nc.vector.reciprocal(var_r[:], var_r[:])   # rstd
                negb_r = p2b.tile([1, 70], F32, name="negbr")
                nc.vector.tensor_mul(negb_r[:], mean_r[:], var_r[:])
                nc.vector.tensor_scalar_mul(negb_r[:], negb_r[:], -1.0)
                rstd_f = stp.tile([1, NPOS], F32, name="rstdf", tag="st")
                negb_f = stp.tile([1, NPOS], F32, name="negbf", tag="st2")
                for p in range(P):
                    nc.vector.tensor_copy(rstd_f[:, p * 70:(p + 1) * 70], var_r[:])
                    nc.vector.tensor_copy(negb_f[:, p * 70:(p + 1) * 70], negb_r[:])
                nc.sync.dma_start(out=sc2[:], in_=rstd_f[0:1, :])
                nc.sync.dma_start(out=sc3[:], in_=negb_f[0:1, :])

            # ---------------- phase 3: normalize, relu, out conv, residual --------
            with (
                tc.tile_pool(name="p3", bufs=1) as p3,
                tc.tile_pool(name="wts3", bufs=2) as wts3,
                tc.tile_pool(name="xin3", bufs=3) as xin3,
                tc.tile_pool(name="ost", bufs=3) as ost,
                tc.tile_pool(name="ps5", bufs=8, space="PSUM") as ps5,
            ):
                rstd_b = p3.tile([128, NPOS], F32, name="rstdb")
                negb_b = p3.tile([128, NPOS], F32, name="negbb")
                nc.sync.dma_start(out=rstd_b[:], in_=bcast_read(sc2, NPOS))
                nc.sync.dma_start(out=negb_b[:], in_=bcast_read(sc3, NPOS))
                rp = [p3.tile([128, CAP * 81], BF, name=f"rp{c}") for c in range(4)]
                for ct in range(4):
                    nc.vector.memset(rp[ct][:], 0.0)
                    vf = virt[ct].rearrange("a b c -> a (b c)")
                    nc.vector.tensor_mul(vf, vf, rstd_b[:])
                    nc.vector.tensor_add(vf, vf, negb_b[:])
                    dst = bass.AP(tensor=rp[ct].tensor, offset=rp[ct].offset + 10,
                                  ap=[rp[ct].ap[0], [9, 7], [1, 7], [81, 70]])
                    src = virt[ct].rearrange("a (y x) i -> a y x i", y=7)
                    nc.scalar.activation(dst, src,
                                         func=mybir.ActivationFunctionType.Relu)
                for cto in range(4):
                    wt = wts3.tile([128, 4, 9, 128], BF, name="wt3", tag="wt3")
                    for ci in range(4):
                        srcap = bass.AP(
                            tensor=wo_d[:].tensor, offset=ci * 589824 + cto * 128,
                            ap=[[4608, 128], [512, 9], [1, 128]])
                        nc.sync.dma_start(out=wt[:, ci, :, :], in_=srcap)
                    for blk in range(NB):
                        xit = xin3.tile([128, 490], F32, name="xi", tag="xi")
                        nc.sync.dma_start(
                            out=xit[:], in_=xint_d[cto][:, blk * 490:(blk + 1) * 490])
                        acc = ps5.tile([128, 490], F32, name="acc3", tag="acc3")
                        fi = True
                        for ci in range(4):
                            for tap in range(9):
                                nc.tensor.matmul(acc[:], wt[:, ci, tap, :],
                                                 conv_rhs(rp[ci], blk, tap),
                                                 start=fi, stop=(ci == 3 and tap == 8))
                                fi = False
                        o = ost.tile([128, 490], F32, name="o", tag="o")
                        nc.vector.tensor_add(o[:], acc[:], xit[:])
                        nc.sync.dma_start(
                            out=y_d[cto][:, blk * 490:(blk + 1) * 490], in_=o[:])

    _split_multiwait(nc)
    _NC_CACHE['nc'] = nc
    return nc


def _shard(rois):
    vid = rois[:, 0].astype(np.int64)
    sizes = np.bincount(vid, minlength=32)
    order = np.argsort(-sizes, kind='stable')
    loads = np.zeros(NCORE, np.int64)
    v2c = np.zeros(32, np.int64)
    for v in order:
        c = int(np.argmin(loads))
        loads[c] += sizes[v]
        v2c[v] = c
    core_of_roi = v2c[vid]
    idxs = [np.nonzero(core_of_roi == c)[0] for c in range(NCORE)]
    for ix in idxs:
        assert len(ix) <= CAP, f"core load {len(ix)} exceeds CAP={CAP}"
    return idxs, vid


def kernel(x, rois, w_q, w_k, w_v, w_out, gamma, beta):
    _install_profhook()
    nc = _build()
    x = np.asarray(x, np.float32)
    rois = np.asarray(rois)
    assert np.allclose(np.asarray(gamma), 1.0) and np.allclose(np.asarray(beta), 0.0), \
        "kernel folds GN affine assuming gamma=1, beta=0"
    idxs, vid = _shard(rois)

    def wprep(w, scale=1.0):
        # [co, ci, 1, 3, 3] -> [ci(4,128), tap, co(4,128)] bf16
        a = (np.asarray(w, np.float32)[:, :, 0] * scale).transpose(1, 2, 3, 0)
        return np.ascontiguousarray(
            a.reshape(4, 128, 9, 4, 128)).astype(ml_dtypes.bfloat16)

    wq = wprep(w_q, 1.0 / np.sqrt(np.float32(C)))
    wk, wv, wo = wprep(w_k), wprep(w_v), wprep(w_out)

    in_maps = []
    for c in range(NCORE):
        ix = idxs[c]
        n = len(ix)
        xpad = np.zeros((CAP, C, 9, 9), np.float32)
        xpad[:n, :, 1:8, 1:8] = x[ix, :, 0]
        xp = np.ascontiguousarray(
            xpad.transpose(1, 0, 2, 3).reshape(4, 128, CAP * 81)
        ).astype(ml_dtypes.bfloat16)
        xi = np.zeros((CAP, C, P), np.float32)
        xi[:n] = x[ix, :, 0].reshape(n, C, P)
        xint = np.ascontiguousarray(xi.transpose(1, 0, 2).reshape(4, 128, NPOS))
        ids = np.full(CAP, -1, np.int64)
        ids[:n] = vid[ix]
        ids[n:] = 1000 + np.arange(CAP - n)
        mask = np.where(ids[:, None] == ids[None, :], 0.0, -1e30).astype(np.float32)
        in_maps.append(dict(xp=xp, xint=xint, wq=wq, wk=wk, wv=wv, wo=wo, mask=mask))

    res = run_bass_kernel_spmd(nc, in_maps, list(range(NCORE)))
    kernel.last_exec_ns = res.exec_time_ns

    out = np.empty((512, C, 1, 7, 7), np.float32)
    for c in range(NCORE):
        ix = idxs[c]
        n = len(ix)
        yc = res.results[c]["y"].reshape(C, CAP, P).transpose(1, 0, 2)
        out[ix] = yc[:n].reshape(n, C, 1, 7, 7)
    return out


# revision 7
# speedup vs baseline: 1.0087x; 1.0029x over previous
"""HR2O_NL sparse-attention kernel for 8 Trainium2 NeuronCores.

Sharding: data-parallel over ROI groups (videos LPT-binpacked onto 8 cores,
whole groups stay local). Conv weights + GN params replicated. Each core runs
q/k/v 3x3 convs (bf16 matmuls, fp32 PSUM), per-position masked attention,
GroupNorm, relu, out-conv, residual — all on its ROI shard (padded to CAP=70).
"""
import sys, types
import numpy as np
import ml_dtypes

import concourse.bass as bass
import concourse.mybir as mybir
import concourse.tile as tile
from concourse.bass_utils import run_bass_kernel_spmd

BF = mybir.dt.bfloat16
F32 = mybir.dt.float32
CAP = 70          # padded ROIs per core
NB = 7            # roi blocks of 10
C = 512
P = 49            # 7x7 positions
NCORE = 8
NPOS = CAP * P    # 3430


def _install_profhook():
    if 'antenv.axon_hooks' in sys.modules:
        return
    try:
        from trn_agent_boot.trn_boot import _ntff_profile_via_ctypes
        hook = _ntff_profile_via_ctypes('/opt/axon/libaxon_pjrt.so')
    except Exception:
        hook = None
    m = types.ModuleType('antenv.axon_hooks')
    m.get_axon_ntff_profile_hook = lambda: hook
    sys.modules['antenv.axon_hooks'] = m


def _walk_blocks(bb):
    yield bb
    for inner in getattr(bb, 'blocks', []) or []:
        yield from _walk_blocks(inner)


def _split_multiwait(nc):
    # this walrus build accepts one sync wait per instruction
    fn = nc.m.functions[0]
    for bb in list(_walk_blocks(fn)):
        insts = getattr(bb, 'instructions', None)
        if not insts:
            continue
        new_list, changed = [], False
        for inst in insts:
            si = inst.sync_info
            if si is not None and si.on_wait is not None and len(si.on_wait) > 1:
                waits = list(si.on_wait)
                for j, w in enumerate(waits[:-1]):
                    d = mybir.InstDrain(name=f"{inst.name}_ws{j}", ins=[], outs=[])
                    d.engine = inst.engine
                    d.sync_info = mybir.SyncInfo(on_wait=[w], on_update=[])
                    new_list.append(d)
                si.on_wait = [waits[-1]]
                changed = True
            new_list.append(inst)
        if changed:
            insts[:] = new_list


_NC_CACHE = {}


def _build():
    if 'nc' in _NC_CACHE:
        return _NC_CACHE['nc']
    nc = bass.Bass("TRN2", target_bir_lowering=False, debug=False, num_devices=NCORE)
    xp_d = nc.dram_tensor("xp", [4, 128, CAP * 81], BF, kind="ExternalInput")
    xint_d = nc.dram_tensor("xint", [4, 128, NPOS], F32, kind="ExternalInput")
    wq_d = nc.dram_tensor("wq", [4, 128, 9, 4, 128], BF, kind="ExternalInput")
    wk_d = nc.dram_tensor("wk", [4, 128, 9, 4, 128], BF, kind="ExternalInput")
    wv_d = nc.dram_tensor("wv", [4, 128, 9, 4, 128], BF, kind="ExternalInput")
    wo_d = nc.dram_tensor("wo", [4, 128, 9, 4, 128], BF, kind="ExternalInput")
    mask_d = nc.dram_tensor("mask", [CAP, CAP], F32, kind="ExternalInput")
    y_d = nc.dram_tensor("y", [4, 128, NPOS], F32, kind="ExternalOutput")
    v_dram = nc.dram_tensor("v_sc", [CAP, 4, 128, P], BF)
    sc1 = nc.dram_tensor("sc1", [NPOS], F32)   # recip bounce
    sc2 = nc.dram_tensor("sc2", [NPOS], F32)   # rstd bounce
    sc3 = nc.dram_tensor("sc3", [NPOS], F32)   # negB bounce

    def conv_rhs(xt, blk, tap):
        dy, dx = tap // 3, tap % 3
        return bass.AP(tensor=xt.tensor, offset=xt.offset + blk * 810 + dy * 9 + dx,
                       ap=[xt.ap[0], [81, 10], [9, 7], [1, 7]])

    def bcast_read(handle, n):
        base = handle[:]
        return bass.AP(tensor=base.tensor, offset=0, ap=[[0, 128], [1, n]])

    with tile.TileContext(nc) as tc:
        with tc.tile_pool(name="persist", bufs=1) as pp:
            attw = pp.tile([70, P, 70], BF, name="attw")
            virt = [pp.tile([128, P, 70], F32, name=f"virt{t}") for t in range(4)]
            qkp_cm = tc.tile_pool(name="qk", bufs=1)
            qkp = qkp_cm.__enter__()
            q_s = [qkp.tile([128, NPOS], BF, name=f"q{t}") for t in range(4)]
            k_s = [qkp.tile([128, NPOS], BF, name=f"k{t}") for t in range(4)]

            # ---------------- phase 1: q,k,v convs ----------------
            with (
                tc.tile_pool(name="p1", bufs=1) as p1,
                tc.tile_pool(name="wts", bufs=2) as wts,
                tc.tile_pool(name="vst", bufs=3) as vst,
                tc.tile_pool(name="ps1", bufs=8, space="PSUM") as ps1,
            ):
                xt = [p1.tile([128, CAP * 81], BF, name=f"xp{c}") for c in range(4)]
                for c in range(4):
                    nc.sync.dma_start(out=xt[c][:], in_=xp_d[c])
                for wd, dst in ((wq_d, q_s), (wk_d, k_s), (wv_d, None)):
                    for cto in range(4):
                        wt = wts.tile([128, 4, 9, 128], BF, name="wt", tag="wt")
                        for ci in range(4):
                            srcap = bass.AP(
                                tensor=wd[:].tensor, offset=ci * 589824 + cto * 128,
                                ap=[[4608, 128], [512, 9], [1, 128]])
                            nc.sync.dma_start(out=wt[:, ci, :, :], in_=srcap)
                        for blk in range(NB):
                            acc = ps1.tile([128, 490], F32, name="acc", tag="acc")
                            fi = True
                            for ci in range(4):
                                for tap in range(9):
                                    nc.tensor.matmul(acc[:], wt[:, ci, tap, :],
                                                     conv_rhs(xt[ci], blk, tap),
                                                     start=fi, stop=(ci == 3 and tap == 8))
                                    fi = False
                            if dst is not None:
                                nc.vector.tensor_copy(
                                    dst[cto][:, blk * 490:(blk + 1) * 490], acc[:])
                            else:
                                vs = vst.tile([128, 490], BF, name="vs", tag="vs")
                                nc.vector.tensor_copy(vs[:], acc[:])
                                dstap = bass.AP(
                                    tensor=v_dram[:].tensor,
                                    offset=(blk * 10 * 4 + cto) * 128 * P,
                                    ap=[[P, 128], [4 * 128 * P, 10], [1, P]])
                                nc.sync.dma_start(out=dstap, in_=vs[:])

            # ---------------- phase 2a: QK^T + mask + exp ----------------
            with (
                tc.tile_pool(name="p2a", bufs=1) as p2a,
                tc.tile_pool(name="ps2", bufs=4, space="PSUM") as ps2,
            ):
                mask_t = p2a.tile([CAP, CAP], F32, name="mask")
                nc.sync.dma_start(out=mask_t[:], in_=mask_d[:])
                mask7 = p2a.tile([70, 7, 70], F32, name="mask7")
                for r in range(7):
                    nc.vector.tensor_copy(mask7[:, r, :], mask_t[:70, :70])
                attf = p2a.tile([70, P, 70], F32, name="attf")
                for pg in range(7):
                    aps = ps2.tile([70, 490], F32, name="aps", tag="aps")
                    for pp in range(7):
                        p = pg * 7 + pp
                        for ct in range(4):
                            lhsT = bass.AP(tensor=k_s[ct].tensor, offset=k_s[ct].offset + p,
                                           ap=[k_s[ct].ap[0], [P, 70]])
                            rhs = bass.AP(tensor=q_s[ct].tensor, offset=q_s[ct].offset + p,
                                          ap=[q_s[ct].ap[0], [P, 70]])
                            nc.tensor.matmul(aps[:, pp * 70:(pp + 1) * 70], lhsT, rhs,
                                             start=(ct == 0), stop=(ct == 3))
                    nc.vector.tensor_add(
                        attf[:, pg * 7:(pg + 1) * 7, :].rearrange("a b c -> a (b c)"),
                        aps[:], mask7.rearrange("a b c -> a (b c)"))
                nc.scalar.activation(
                    attw.rearrange("a b c -> a (b c)"),
                    attf.rearrange("a b c -> a (b c)"),
                    func=mybir.ActivationFunctionType.Exp)
            qkp_cm.__exit__(None, None, None)

            # ---------------- phase 2b: rowsum, AV, divide ----------------
            with (
                tc.tile_pool(name="p2b", bufs=1) as p2b,
                tc.tile_pool(name="stats", bufs=1) as stp,
                tc.tile_pool(name="ps3", bufs=4, space="PSUM") as ps3,
                tc.tile_pool(name="ps4", bufs=2, space="PSUM") as ps4,
            ):
                ones_t = p2b.tile([70, 1], BF, name="ones")
                nc.vector.memset(ones_t[:], 1.0)
                rsum = stp.tile([1, NPOS], F32, name="rsum", tag="st")
                for pc in range(7):
                    op = ps4.tile([1, 490], F32, name="op", tag="op")
                    nc.tensor.matmul(
                        op[:], ones_t[:],
                        attw[:, pc * 7:(pc + 1) * 7, :].rearrange("a b c -> a (b c)"),
                        start=True, stop=True)
                    nc.vector.tensor_copy(rsum[:, pc * 490:(pc + 1) * 490], op[:])
                nc.vector.reciprocal(rsum[:], rsum[:])
                nc.sync.dma_start(out=sc1[:], in_=rsum[0:1, :])
                recip_b = p2b.tile([128, NPOS], F32, name="recipb")
                nc.sync.dma_start(out=recip_b[:], in_=bcast_read(sc1, NPOS))
                vth = p2b.tile([70, 2, 128, P], BF, name="vth", tag="vth")
                for h in range(2):
                    if h == 1:
                        vth = p2b.tile([70, 2, 128, P], BF, name="vth2", tag="vth")
                    nc.sync.dma_start(out=vth[:], in_=v_dram[:, h * 2:h * 2 + 2])
                    for ctp in range(2):
                        ct = h * 2 + ctp
                        for pg in range(7):
                            av = ps3.tile([128, 490], F32, name="av", tag="av")
                            for pp in range(7):
                                p = pg * 7 + pp
                                nc.tensor.matmul(av[:, pp * 70:(pp + 1) * 70],
                                                 vth[:, ctp, :, p], attw[:, p, :],
                                                 start=True, stop=True)
                            nc.vector.tensor_copy(
                                virt[ct][:, pg * 7:(pg + 1) * 7, :].rearrange("a b c -> a (b c)"),
                                av[:])
                for ct in range(4):
                    vf = virt[ct].rearrange("a b c -> a (b c)")
                    nc.vector.tensor_mul(vf, vf, recip_b[:])

                # ---- GroupNorm stats (per-i over c,p) ----
                vbfp = p2b.tile([128, NPOS], BF, name="vbf", tag="vbf")
                s1 = stp.tile([1, NPOS], F32, name="s1", tag="st")
                s2 = stp.tile([1, NPOS], F32, name="s2", tag="st2")
                onesf = p2b.tile([128, 1], BF, name="onesf")
                nc.vector.memset(onesf[:], 1.0)
                for which, sdst in ((0, s1), (1, s2)):
                    for chunk in range(7):
                        op = ps4.tile([1, 490], F32, name="op2", tag="op")
                        for ct in range(4):
                            vf = virt[ct].rearrange("a b c -> a (b c)")
                            seg = vf[:, chunk * 490:(chunk + 1) * 490]
                            if which == 0:
                                nc.vector.tensor_copy(vbfp[:, chunk * 490:(chunk + 1) * 490], seg)
                            else:
                                nc.vector.tensor_mul(vbfp[:, chunk * 490:(chunk + 1) * 490], seg, seg)
                            nc.tensor.matmul(op[:], onesf[:],
                                             vbfp[:, chunk * 490:(chunk + 1) * 490],
                                             start=(ct == 0), stop=(ct == 3))
                        nc.vector.tensor_copy(sdst[:, chunk * 490:(chunk + 1) * 490], op[:])

                s1i = p2b.tile([1, 70], F32, name="s1i")
                s2i = p2b.tile([1, 70], F32, name="s2i")
                for src, dsti in ((s1, s1i), (s2, s2i)):
                    v3 = bass.AP(tensor=src.tensor, offset=src.offset,
                                 ap=[src.ap[0], [1, 70], [70, P]])
                    nc.vector.reduce_sum(dsti[:], v3, axis=mybir.AxisListType.X)
                inv_n = 1.0 / (C * P)
                mean_r = p2b.tile([1, 70], F32, name="meanr")
                var_r = p2b.tile([1, 70], F32, name="varr")
                nc.vector.tensor_scalar_mul(mean_r[:], s1i[:], inv_n)
                nc.vector.tensor_scalar_mul(var_r[:], s2i[:], inv_n)
                msq = p2b.tile([1, 70], F32, name="msq")
                nc.vector.tensor_mul(msq[:], mean_r[:], mean_r[:])
                nc.vector.tensor_sub(var_r[:], var_r[:], msq[:])
                eps_t = p2b.tile([1, 1], F32, name="eps")
                nc.vector.memset(eps_t[:], 1e-5)
                nc.scalar.activation(var_r[:], var_r[:],
                                     func=mybir.ActivationFunctionType.Sqrt,
                                     bias=eps_t[:], scale=1.0)
                nc.vector.reciprocal(var_r[:], var_r[:])   # rstd
                negb_r = p2b.tile([1, 70], F32, name="negbr")
                nc.vector.tensor_mul(negb_r[:], mean_r[:], var_r[:])
                nc.vector.tensor_scalar_mul(negb_r[:], negb_r[:], -1.0)
                rstd_f = stp.tile([1, NPOS], F32, name="rstdf", tag="st")
                negb_f = stp.tile([1, NPOS], F32, name="negbf", tag="st2")
                for p in range(P):
                    nc.vector.tensor_copy(rstd_f[:, p * 70:(p + 1) * 70], var_r[:])
                    nc.vector.tensor_copy(negb_f[:, p * 70:(p + 1) * 70], negb_r[:])
                nc.sync.dma_start(out=sc2[:], in_=rstd_f[0:1, :])
                nc.sync.dma_start(out=sc3[:], in_=negb_f[0:1, :])

            # ---------------- phase 3: normalize, relu, out conv, residual --------
            with (
                tc.tile_pool(name="p3", bufs=1) as p3,
                tc.tile_pool(name="wts3", bufs=2) as wts3,
                tc.tile_pool(name="xin3", bufs=3) as xin3,
                tc.tile_pool(name="ost", bufs=3) as ost,
                tc.tile_pool(name="ps5", bufs=8, space="PSUM") as ps5,
            ):
                rstd_b = p3.tile([128, NPOS], F32, name="rstdb")
                negb_b = p3.tile([128, NPOS], F32, name="negbb")
                nc.sync.dma_start(out=rstd_b[:], in_=bcast_read(sc2, NPOS))
                nc.sync.dma_start(out=negb_b[:], in_=bcast_read(sc3, NPOS))
                rp = [p3.tile([128, CAP * 81], BF, name=f"rp{c}") for c in range(4)]
                for ct in range(4):
                    nc.vector.memset(rp[ct][:], 0.0)
                    vf = virt[ct].rearrange("a b c -> a (b c)")
                    nc.vector.tensor_mul(vf, vf, rstd_b[:])
                    nc.vector.tensor_add(vf, vf, negb_b[:])
                    dst = bass.AP(tensor=rp[ct].tensor, offset=rp[ct].offset + 10,
                                  ap=[rp[ct].ap[0], [9, 7], [1, 7], [81, 70]])
                    src = virt[ct].rearrange("a (y x) i -> a y x i", y=7)
                    nc.scalar.activation(dst, src,
                                         func=mybir.ActivationFunctionType.Relu)
                for cto in range(4):
                    wt = wts3.tile([128, 4, 9, 128], BF, name="wt3", tag="wt3")
                    for ci in range(4):
                        srcap = bass.AP(
                            tensor=wo_d[:].tensor, offset=ci * 589824 + cto * 128,
                            ap=[[4608, 128], [512, 9], [1, 128]])
                        nc.sync.dma_start(out=wt[:, ci, :, :], in_=srcap)
                    for blk in range(NB):
                        xit = xin3.tile([128, 490], F32, name="xi", tag="xi")
                        nc.sync.dma_start(
                            out=xit[:], in_=xint_d[cto][:, blk * 490:(blk + 1) * 490])
                        acc = ps5.tile([128, 490], F32, name="acc3", tag="acc3")
                        fi = True
                        for ci in range(4):
                            for tap in range(9):
                                nc.tensor.matmul(acc[:], wt[:, ci, tap, :],
                                                 conv_rhs(rp[ci], blk, tap),
                                                 start=fi, stop=(ci == 3 and tap == 8))
                                fi = False
                        o = ost.tile([128, 490], F32, name="o", tag="o")
                        nc.vector.tensor_add(o[:], acc[:], xit[:])
                        nc.sync.dma_start(
                            out=y_d[cto][:, blk * 490:(blk + 1) * 490], in_=o[:])

    _split_multiwait(nc)
    _NC_CACHE['nc'] = nc
    return nc


def _shard(rois):
    vid = rois[:, 0].astype(np.int64)
    sizes = np.bincount(vid, minlength=32)
    order = np.argsort(-sizes, kind='stable')
    loads = np.zeros(NCORE, np.int64)
    v2c = np.zeros(32, np.int64)
    for v in order:
        c = int(np.argmin(loads))
        loads[c] += sizes[v]
        v2c[v] = c
    core_of_roi = v2c[vid]
    idxs = [np.nonzero(core_of_roi == c)[0] for c in range(NCORE)]
    for ix in idxs:
        assert len(ix) <= CAP, f"core load {len(ix)} exceeds CAP={CAP}"
    return idxs, vid


def kernel(x, rois, w_q, w_k, w_v, w_out, gamma, beta):
    _install_profhook()
    nc = _build()
    x = np.asarray(x, np.float32)
    rois = np.asarray(rois)
    assert np.allclose(np.asarray(gamma), 1.0) and np.allclose(np.asarray(beta), 0.0), \
        "kernel folds GN affine assuming gamma=1, beta=0"
    idxs, vid = _shard(rois)

    def wprep(w, scale=1.0):
        # [co, ci, 1, 3, 3] -> [ci(4,128), tap, co(4,128)] bf16
        a = (np.asarray(w, np.float32)[:, :, 0] * scale).transpose(1, 2, 3, 0)
        return np.ascontiguousarray(
            a.reshape(4, 128, 9, 4, 128)).astype(ml_dtypes.bfloat16)

    wq = wprep(w_q, 1.0 / np.sqrt(np.float32(C)))
    wk, wv, wo = wprep(w_k), wprep(w_v), wprep(w_out)

    in_maps = []
    for c in range(NCORE):
        ix = idxs[c]
        n = len(ix)
        xpad = np.zeros((CAP, C, 9, 9), np.float32)
        xpad[:n, :, 1:8, 1:8] = x[ix, :, 0]
        xp = np.ascontiguousarray(
            xpad.transpose(1, 0, 2, 3).reshape(4, 128, CAP * 81)
        ).astype(ml_dtypes.bfloat16)
        xi = np.zeros((CAP, C, P), np.float32)
        xi[:n] = x[ix, :, 0].reshape(n, C, P)
        xint = np.ascontiguousarray(xi.transpose(1, 0, 2).reshape(4, 128, NPOS))
        ids = np.full(CAP, -1, np.int64)
        ids[:n] = vid[ix]
        ids[n:] = 1000 + np.arange(CAP - n)
        mask = np.where(ids[:, None] == ids[None, :], 0.0, -1e30).astype(np.float32)
        in_maps.append(dict(xp=xp, xint=xint, wq=wq, wk=wk, wv=wv, wo=wo, mask=mask))

    res = run_bass_kernel_spmd(nc, in_maps, list(range(NCORE)))
    kernel.last_exec_ns = res.exec_time_ns

    out = np.empty((512, C, 1, 7, 7), np.float32)
    for c in range(NCORE):
        ix = idxs[c]
        n = len(ix)
        yc = res.results[c]["y"].reshape(C, CAP, P).transpose(1, 0, 2)
        out[ix] = yc[:n].reshape(n, C, 1, 7, 7)
    return out
